# revision 9
# baseline (speedup 1.0000x reference)
"""DDSP core synthesizer kernel for Trainium2 (8 NeuronCores, data-parallel).

Reference computation (per row of B=32, T=64000):
    f0_hz = 20*exp(f0); phase = cumsum(2*pi*f0_hz/SR)
    hw    = sum_k sin(phase*k)/k   (k = 1..60)
    audio = mix*hw*loud + (1-mix)*noise*loud;  out = audio / (max|audio| + 1e-6)

Device algorithm (everything in "turns" = phase/2pi):
    inc  = exp(f0 + ln(20/SR))                       [ACT Exp]
    u    = blocked cumsum of inc                      [DVE scan + PE triangular mm]
    u1   = u - rint(u)                                [DVE magic-number rint]
    u_hi = f16(u1); u_lo = f16((u1-u_hi)*4096)        [exact 2-term split]
    per (group, section, kg) unit of 512 cols (2 harmonics x 64 blocks):
        x  = k*u_hi + (k/4096)*u_lo                   [PE f16 matmul, exact]
        r  = rint(x)                                  [DVE dual-op tensor_scalar]
        v  = x - r   in [-0.5, 0.5]                   [PE -identity @ r accumulate]
        s  = sin(2*pi*v)                              [ACT Sin, scale=2pi]
        hw += (1/k)^T @ s                             [PE f16 matmul accumulate]
    The PE instruction stream is software-pipelined (x at unit i, the
    fractional-subtract at i-1, the harmonic-sum at i-2) so a semaphore wait
    on one stage never head-of-line-blocks the next unit's x matmuls.
    epilogue per (g, sec): audio = ln + ml*(hw - noise) on GpSimd with
    ml = mix*loud, ln = noise*loud precomputed; per-row peak via free-dim
    abs-max partials + DVE 32x32 transpose trick.

Sharding: pure data parallel, 4 rows per core, SPMD on cores 0-7.
"""

import sys

sys.path.insert(0, "/opt/trn_rl_repo")

import numpy as np
import ml_dtypes
from contextlib import ExitStack

import concourse.bass as bass
import concourse.tile as tile
from concourse import bacc, mybir
from concourse import bass_utils

f32 = np.float32
dt = mybir.dt

SR = 44100.0
H = 60                      # harmonics
B, T = 32, 64000
NCORES = 8
RPC = B // NCORES           # rows per core = 4
P = 128                     # SBUF partitions
FD = T * RPC // P           # free dim of master tiles = 2000
BPR = P // RPC              # blocks per row = 32
PI = float(np.pi)
MAGIC = float(1.5 * 2.0 ** 23)
LO_SCALE = 4096.0
Q_OFFS = [0, 512, 1024, 1536]
Q_LENS = [512, 512, 512, 464]
EXP_BIAS = float(np.log(20.0 / SR))

_cache = {}


def _consts():
    # lt: exclusive-prefix matmul weights. offs[m] = sum_k lt[k, m] * totals[k]
    kk, mm_ = np.meshgrid(np.arange(P), np.arange(P), indexing="ij")
    lt = ((kk // BPR == mm_ // BPR) & (kk % BPR < mm_ % BPR)).astype(f32)

    # Stage-2 partitioning: per pass, 64 local blocks x 2 harmonics fill
    # 128 partitions (p = 2*b_loc + kap, k = 2*kg + kap + 1; kg = 0..29).
    # Group tile uhalf[g] holds local block b at partitions 2b (hi), 2b+1 (lo).
    # xsel[kg]: lhsT [128, 128], x[2b+kap] = k*u_hi[b] + (k/4096)*u_lo[b].
    # wsel[kg]: lhsT [128, 64], hw[b] += sum_kap (1/k) * s[2b+kap].
    xsel = np.zeros((30, P, P), dtype=np.float64)
    wsel = np.zeros((30, P, 64), dtype=f32)  # cast at return
    negi = np.zeros((P, P), dtype=np.float64)
    for p in range(P):
        negi[p, p] = -1.0
    for kg in range(30):
        for b in range(64):
            for kap in range(2):
                k = 2 * kg + kap + 1.0
                xsel[kg, 2 * b + 0, 2 * b + kap] = k
                xsel[kg, 2 * b + 1, 2 * b + kap] = k / LO_SCALE
                wsel[kg, 2 * b + kap, b] = 1.0 / k
    xsel = xsel.astype(np.float16)
    negi = negi.astype(np.float16)
    wsel = wsel.astype(np.float16)
    return {"lt": lt, "xsel": xsel, "wsel": wsel, "negi": negi}


def _build(xbufs=6, rsbufs=5, hbufs=2, njunk=110, s1chunks=4):
    nc = bacc.Bacc("TRN2", target_bir_lowering=False, debug=False,
                   enable_asserts=True, num_devices=NCORES)

    f0_d = nc.dram_tensor("f0", [P, FD], dt.float32, kind="ExternalInput")
    loud_d = nc.dram_tensor("loud", [P, FD], dt.float32, kind="ExternalInput")
    mix_d = nc.dram_tensor("mix", [P, FD], dt.float32, kind="ExternalInput")
    noise_d = nc.dram_tensor("noise", [P, FD], dt.float32, kind="ExternalInput")
    lt_d = nc.dram_tensor("lt", [P, P], dt.float32, kind="ExternalInput")
    xsel_d = nc.dram_tensor("xsel", [30, P, P], dt.float16, kind="ExternalInput")
    wsel_d = nc.dram_tensor("wsel", [30, P, 64], dt.float16, kind="ExternalInput")
    negi_d = nc.dram_tensor("negi", [P, P], dt.float16, kind="ExternalInput")
    out_d = nc.dram_tensor("audio", [P, FD], dt.float32, kind="ExternalOutput")

    AF = mybir.ActivationFunctionType
    ALU = mybir.AluOpType

    with tile.TileContext(nc) as tc, ExitStack() as ctx:
        pool = ctx.enter_context(tc.tile_pool(name="sb", bufs=1))
        rpool = ctx.enter_context(tc.tile_pool(name="rint", bufs=rsbufs))
        spool = ctx.enter_context(tc.tile_pool(name="sin", bufs=rsbufs))
        epool = ctx.enter_context(tc.tile_pool(name="epi", bufs=3))
        xpool = ctx.enter_context(tc.tile_pool(name="xps", bufs=xbufs, space="PSUM"))
        hpool = ctx.enter_context(tc.tile_pool(name="hps", bufs=hbufs, space="PSUM"))

        def const_col(val, tag):
            t = pool.tile([P, 1], dt.float32, tag=tag)
            nc.vector.memset(t[:], val)
            return t

        exp_bias = const_col(EXP_BIAS, "cbias_exp")
        zero_bias = const_col(0.0, "cbias_zero")

        # ---- input DMA: f0 first and alone on the sync queue so its HBM
        # transfer isn't starved by the large const tensors; consts follow
        # on the same queue (their transfers serialize behind f0). The
        # epilogue inputs go on the gpsimd queue and trickle in later. ----
        f0 = pool.tile([P, FD], dt.float32, tag="scr", bufs=4, name="f0")
        nc.sync.dma_start(f0[:], f0_d.ap())
        lt = pool.tile([P, P], dt.float32)
        nc.sync.dma_start(lt[:], lt_d.ap())
        negi = pool.tile([P, P], dt.float16)
        nc.sync.dma_start(negi[:], negi_d.ap())
        xsel = pool.tile([P, 30, P], dt.float16)
        nc.sync.dma_start(xsel[:], xsel_d.ap().rearrange("g p m -> p g m"))
        wsel = pool.tile([P, 30, 64], dt.float16)
        nc.sync.dma_start(wsel[:], wsel_d.ap().rearrange("g p m -> p g m"))

        louds, mixs, noises = [], [], []
        for g in range(2):
            lg = pool.tile([64, FD], dt.float32, tag=f"loud{g}", name=f"lg{g}")
            nc.gpsimd.dma_start(lg[:], loud_d.ap()[64 * g:64 * g + 64, :])
            louds.append(lg)
            mg = pool.tile([64, FD], dt.float32, tag=f"mix{g}", name=f"mg{g}")
            nc.gpsimd.dma_start(mg[:], mix_d.ap()[64 * g:64 * g + 64, :])
            mixs.append(mg)
            ng = pool.tile([64, FD], dt.float32, tag=f"noise{g}", name=f"ng{g}")
            nc.gpsimd.dma_start(ng[:], noise_d.ap()[64 * g:64 * g + 64, :])
            noises.append(ng)

        # ---- PE warmup: junk matmuls keep the HAM clock gate open through
        # the stage-1 prefix so stage-2 starts at 2.4 GHz ----
        jw = pool.tile([P, 64], dt.float16)
        nc.vector.memset(jw[:], 0.0)
        junk_ps = xpool.tile([P, 512], dt.float32, tag="x", name="junk")
        for _ in range(njunk):
            nc.tensor.matmul(junk_ps[0:64, 0:64], jw[:], jw[:],
                             start=True, stop=True)

        # ---- stage 1: phase accumulation (turns) ----
        inc = pool.tile([P, FD], dt.float32, tag="scr", bufs=4, name="inc")
        nc.scalar.activation(inc[:], f0[:], AF.Exp, bias=exp_bias[:, 0:1], scale=1.0)

        local = pool.tile([P, FD], dt.float32, tag="scr", bufs=4, name="local")
        nc.vector.tensor_tensor_scan(local[:], inc[:], inc[:], 0.0,
                                     ALU.add, ALU.bypass)

        offs_ps = xpool.tile([P, 512], dt.float32, tag="x", name="offs_ps")
        nc.tensor.matmul(offs_ps[:, 0:1], lt[:], local[:, FD - 1:FD],
                         start=True, stop=True)
        offs = pool.tile([P, 1], dt.float32)
        nc.vector.tensor_copy(offs[:], offs_ps[:, 0:1])

        # chunked tail of stage 1 so stage 2 can start after the first chunk
        uhalf = []
        for g in range(2):
            t = pool.tile([P, FD], dt.float16, tag=f"uhl{g}", name=f"uhalf{g}")
            uhalf.append(t)
        cw = FD // s1chunks
        for c in range(s1chunks):
            cs = slice(c * cw, (c + 1) * cw)
            u = pool.tile([P, cw], dt.float32, tag="s1u", bufs=2, name="u")
            nc.vector.tensor_scalar(u[:], local[:, cs], offs[:, 0:1], None,
                                    ALU.add)
            ur = pool.tile([P, cw], dt.float32, tag="s1ur", bufs=2, name="ur")
            nc.vector.tensor_scalar(ur[:], u[:], MAGIC, MAGIC,
                                    ALU.add, ALU.subtract)
            u1 = pool.tile([P, cw], dt.float32, tag="s1u1", bufs=2, name="u1")
            nc.vector.tensor_tensor(u1[:], u[:], ur[:], ALU.subtract)
            uhi = pool.tile([P, cw], dt.float16, tag="s1uhi", bufs=2, name="uhi")
            nc.vector.tensor_copy(uhi[:], u1[:])
            ulo32 = pool.tile([P, cw], dt.float32, tag="s1ulo32", bufs=2,
                              name="ulo32")
            nc.vector.tensor_tensor(ulo32[:], u1[:], uhi[:], ALU.subtract)
            ulo = pool.tile([P, cw], dt.float16, tag="s1ulo", bufs=2, name="ulo")
            nc.vector.tensor_scalar(ulo[:], ulo32[:], LO_SCALE, None, ALU.mult)
            uhi_v = uhi[:].rearrange("(g b) f -> g b f", g=2)
            ulo_v = ulo[:].rearrange("(g b) f -> g b f", g=2)
            for g in range(2):
                tv = uhalf[g][:].rearrange("(b s) f -> b s f", s=2)
                nc.sync.dma_start(tv[:, 0, cs], uhi_v[g])
                nc.scalar.dma_start(tv[:, 1, cs], ulo_v[g])

        # epilogue precomputes (GpSimd is idle during the k-loop)
        mls, lns = [], []
        for g in range(2):
            ml = pool.tile([64, FD], dt.float32, tag=f"ml{g}", name=f"ml{g}")
            nc.gpsimd.tensor_tensor(ml[:], mixs[g][:], louds[g][:], ALU.mult)
            mls.append(ml)
            ln = pool.tile([64, FD], dt.float32, tag=f"ln{g}", name=f"ln{g}")
            nc.gpsimd.tensor_tensor(ln[:], noises[g][:], louds[g][:], ALU.mult)
            lns.append(ln)

        audios = [pool.tile([64, FD], dt.float32, tag=f"audio{g}",
                            name=f"audio{g}") for g in range(2)]
        pkps = [pool.tile([64, 4], dt.float32, tag=f"pkp{g}", name=f"pkp{g}")
                for g in range(2)]

        # ---- stage 2: software-pipelined unit loop ----
        # unit = (g, sec, kg): 512 cols, 2 harmonics, 64 blocks
        units = [(g, sec, kg)
                 for g in range(2) for sec in range(4) for kg in range(30)]
        NU = len(units)
        xts, rts, sts, hwts = {}, {}, {}, {}

        def emit_front(i):
            g, sec, kg = units[i]
            q0, qn = Q_OFFS[sec], Q_LENS[sec]
            xt = xpool.tile([P, 512], dt.float32, tag="x", name=f"x{i % xbufs}")
            xts[i] = xt
            nc.tensor.matmul(xt[:, 0:qn], xsel[:, kg, :],
                             uhalf[g][:, q0:q0 + qn], start=True, stop=False)
            r = rpool.tile([P, 512], dt.float16, tag="r")
            rts[i] = r
            nc.vector.tensor_scalar(r[:, 0:qn], xt[:, 0:qn], MAGIC, MAGIC,
                                    ALU.add, ALU.subtract)

        def emit_mid(i):
            g, sec, kg = units[i]
            qn = Q_LENS[sec]
            xt, r = xts[i], rts.pop(i)
            nc.tensor.matmul(xt[:, 0:qn], negi[:], r[:, 0:qn],
                             start=False, stop=True)
            s = spool.tile([P, 512], dt.float16, tag="s")
            sts[i] = s
            nc.scalar.activation(s[:, 0:qn], xt[:, 0:qn], AF.Sin,
                                 bias=zero_bias[:, 0:1], scale=2.0 * PI)

        def emit_back(i):
            g, sec, kg = units[i]
            q0, qn = Q_OFFS[sec], Q_LENS[sec]
            s = sts.pop(i)
            xts.pop(i)
            if kg == 0:
                hwts[(g, sec)] = hpool.tile([64, 512], dt.float32, tag="hw",
                                            name=f"hw{(g * 4 + sec) % hbufs}")
            nc.tensor.matmul(hwts[(g, sec)][:, 0:qn], wsel[:, kg, :],
                             s[:, 0:qn], start=(kg == 0), stop=(kg == 29))
            if kg == 29:
                emit_epilogue(g, sec)

        def emit_epilogue(g, sec):
            q0, qn = Q_OFFS[sec], Q_LENS[sec]
            sl = slice(q0, q0 + qn)
            hw_t = hwts.pop((g, sec))
            e1 = epool.tile([64, 512], dt.float32, tag="e1", name="e1")
            nc.vector.tensor_tensor(e1[:, 0:qn], hw_t[:, 0:qn],
                                    noises[g][:, sl], ALU.subtract)
            e2 = epool.tile([64, 512], dt.float32, tag="e2", name="e2")
            nc.gpsimd.tensor_tensor(e2[:, 0:qn], e1[:, 0:qn],
                                    mls[g][:, sl], ALU.mult)
            nc.gpsimd.tensor_tensor(audios[g][:, sl], e2[:, 0:qn],
                                    lns[g][:, sl], ALU.add)
            nc.vector.tensor_reduce(pkps[g][:, sec:sec + 1], audios[g][:, sl],
                                    axis=mybir.AxisListType.X, op=ALU.max,
                                    apply_absolute_value=True)
            if sec == 3:
                emit_final(g)

        def emit_final(g):
            # per-row peak over the 32 blocks of each row: free-dim partials
            # are in pkps[g]; combine, then 32x32 transpose trick
            pkm = pool.tile([64, 1], dt.float32, tag=f"pkm{g}", name=f"pkm{g}")
            nc.vector.tensor_reduce(pkm[:], pkps[g][:, 0:4],
                                    axis=mybir.AxisListType.X, op=ALU.max)
            pkr = pool.tile([64, 32], dt.float32, tag=f"pkr{g}", name=f"pkr{g}")
            nc.vector.tensor_copy(pkr[:], pkm[:, 0:1].to_broadcast((64, 32)))
            pkt = pool.tile([64, 32], dt.float32, tag=f"pkt{g}", name=f"pkt{g}")
            nc.vector.transpose(pkt[:], pkr[:])
            rowmax = pool.tile([64, 1], dt.float32, tag=f"rm{g}", name=f"rm{g}")
            nc.vector.tensor_reduce(rowmax[:], pkt[:],
                                    axis=mybir.AxisListType.X, op=ALU.max)
            rcp = pool.tile([64, 1], dt.float32, tag=f"rcp{g}", name=f"rcp{g}")
            pke = pool.tile([64, 1], dt.float32, tag=f"pke{g}", name=f"pke{g}")
            nc.vector.tensor_scalar(pke[:], rowmax[:], 1e-6, None, ALU.add)
            nc.vector.reciprocal(rcp[:], pke[:])
            half = FD // 2
            for c in range(2):
                cs = slice(c * half, (c + 1) * half)
                outt = epool.tile([64, half], dt.float32, tag="outt",
                                  name="outt")
                nc.vector.tensor_scalar(outt[:], audios[g][:, cs],
                                        rcp[:, 0:1], None, ALU.mult)
                nc.sync.dma_start(out_d.ap()[64 * g:64 * g + 64, cs], outt[:])

        for i in range(NU + 2):
            if i < NU:
                emit_front(i)
            if 1 <= i <= NU:
                emit_mid(i - 1)
            if i >= 2:
                emit_back(i - 2)

    nc.compile()
    return nc


def kernel(f0, loudness, harmonic_mix, noise):
    if "nc" not in _cache:
        _cache["nc"] = _build()
        _cache["consts"] = _consts()
    nc = _cache["nc"]
    consts = _cache["consts"]

    def shard(a, c):
        return np.ascontiguousarray(
            a[c * RPC:(c + 1) * RPC].astype(f32, copy=False).reshape(P, FD))

    in_maps = []
    for c in range(NCORES):
        in_maps.append({
            "f0": shard(f0, c),
            "loud": shard(loudness, c),
            "mix": shard(harmonic_mix, c),
            "noise": shard(noise, c),
            **consts,
        })

    res = bass_utils.run_bass_kernel_spmd(nc, in_maps, core_ids=list(range(NCORES)))
    outs = [res.results[c]["audio"].reshape(RPC, T) for c in range(NCORES)]
    return np.concatenate(outs, axis=0)
